# revision 2
# baseline (speedup 1.0000x reference)
"""Causal multi-head attention on 8 Trainium2 NeuronCores — v3.

Problem: B=4, S=2048, D=1024, H=16, Dk=64, fp32, causal, all-ones padding mask.

Sharding: core = (batch b, head-group g) on a 4x2 grid; each core computes the
8 heads of group g for batch b plus the partial output projection
o @ w_o[:, g-slice].T; the host sums the two partials per batch and adds b_o.

Dataflow:
  - attention uses pexp as the matmul STATIONARY ([keys, queries]) and the
    augmented V ([keys, 65], ones column last) as the MOVING operand: each
    128x128 attention block streams 65 rows. Output lands in natural
    [queries, dims] PSUM layout with the softmax denominator in the ones
    column -> normalization is a per-partition DVE reciprocal+multiply.
  - normalized outputs are PE-transposed to [dims, queries] and the output
    projection runs progressively per 256-query chunk.
  - q/k are bf16 with 1/sqrt(dk) folded into the host-prepped weights.
  - two key tiles share one scores-PSUM tile so exp runs 1024 wide.
  - QK/V projections for chunk c+1 and V tiles are emitted as filler inside
    chunk c's attention j-loops to keep the PE busy while the Activation
    engine (the attention inner-loop bottleneck) chews exp.
"""

import numpy as np
from contextlib import ExitStack

import ml_dtypes

P = 128
S = 2048
D = 1024
DK = 64
HLOC = 8          # heads per core
NPAIR = 4         # head pairs (128 rows of qT/kT each)
CH = 256          # query-chunk width
NCH = S // CH     # 8 chunks
IC = 8            # 128-deep contraction chunks of D
NT = 16           # 128-tall seq tiles

_PROGRAM_CACHE = {}


def build_program(reps=1):
    import concourse.bacc as bacc
    import concourse.bass as bass
    import concourse.mybir as mybir
    import concourse.tile as tile

    f32 = mybir.dt.float32
    bf16 = mybir.dt.bfloat16
    fp8 = mybir.dt.float8e4
    AF = mybir.ActivationFunctionType
    DR = mybir.MatmulPerfMode.DoubleRow

    nc = bacc.Bacc("TRN2", target_bir_lowering=False, debug=False)

    xt = nc.dram_tensor("xt", [D, S], bf16, kind="ExternalInput").ap()
    wvt = nc.dram_tensor("wvt", [D, 512], bf16, kind="ExternalInput").ap()
    wqt = nc.dram_tensor("wqt", [D, 512], bf16, kind="ExternalInput").ap()
    wkt = nc.dram_tensor("wkt", [D, 512], bf16, kind="ExternalInput").ap()
    wot = nc.dram_tensor("wot", [512, D], bf16, kind="ExternalInput").ap()
    bqs = nc.dram_tensor("bqs", [P, NPAIR], f32, kind="ExternalInput").ap()
    bks = nc.dram_tensor("bks", [P, NPAIR], f32, kind="ExternalInput").ap()
    bvb = nc.dram_tensor("bvb", [P, 512], f32, kind="ExternalInput").ap()
    out = nc.dram_tensor("out", [S, D], f32, kind="ExternalOutput").ap()

    with tile.TileContext(nc) as tc:
      for _rep in range(reps):
        sfx = f"_r{_rep}" if reps > 1 else ""
        with ExitStack() as ctx0:
            consts = ctx0.enter_context(tc.tile_pool(name="consts" + sfx, bufs=1))
            vp = ctx0.enter_context(tc.tile_pool(name="vp" + sfx, bufs=1))
            qk = ctx0.enter_context(tc.tile_pool(name="qk" + sfx, bufs=1))
            wqk = ctx0.enter_context(tc.tile_pool(name="wqk" + sfx, bufs=1))
            pexp_pool = ctx0.enter_context(tc.tile_pool(name="pexp" + sfx, bufs=4))
            onorm_pool = ctx0.enter_context(tc.tile_pool(name="onorm" + sfx, bufs=6))
            recip_pool = ctx0.enter_context(tc.tile_pool(name="recip" + sfx, bufs=4))
            oTp_pool = ctx0.enter_context(tc.tile_pool(name="oTp" + sfx, bufs=1))
            stage_pool = ctx0.enter_context(tc.tile_pool(name="stage" + sfx, bufs=4))

            # ---- constants
            bq_sb = consts.tile([P, NPAIR], f32, tag="bq", name="bq" + sfx)
            bk_sb = consts.tile([P, NPAIR], f32, tag="bk", name="bk" + sfx)
            bv_sb = consts.tile([P, 512], f32, tag="bv", name="bv" + sfx)
            tri = consts.tile([P, P], bf16, tag="tri", name="tri" + sfx)
            ident = consts.tile([P, P], bf16, tag="ident", name="ident" + sfx)
            nc.sync.dma_start(bq_sb[:], bqs)
            nc.sync.dma_start(bk_sb[:], bks)
            nc.sync.dma_start(bv_sb[:], bvb)
            # triangle mask: keep col >= row (query >= key), zero below
            nc.vector.memset(tri[:], 1.0)
            nc.gpsimd.affine_select(
                out=tri[:], in_=tri[:],
                compare_op=mybir.AluOpType.is_ge,
                fill=0.0, base=0, pattern=[[1, P]], channel_multiplier=-1,
            )
            nc.gpsimd.memset(ident[:], 0.0)
            nc.gpsimd.affine_select(
                out=ident[:], in_=ident[:],
                compare_op=mybir.AluOpType.not_equal,
                fill=1.0, base=0, pattern=[[-1, P]], channel_multiplier=1,
            )

            # ---- persistent tensors
            xT_sb = consts.tile([P, IC, S], bf16, tag="xT", name="xT" + sfx)
            wv_sb = consts.tile([P, IC, 512], bf16, tag="wv", name="wv" + sfx)
            woT_sb = consts.tile([P, NPAIR, D], bf16, tag="woT", name="woT" + sfx)
            v_sb = [
                vp.tile([P, HLOC, DK + 1], bf16, tag=f"v{t}", name=f"v{t}" + sfx)
                for t in range(NT)
            ]
            qT = {}
            kT = {}
            for m in range(NPAIR):
                qT[m] = qk.tile([P, S], bf16, tag=f"qT{m}", name=f"qT{m}" + sfx)
                kT[m] = qk.tile([P, S], bf16, tag=f"kT{m}", name=f"kT{m}" + sfx)
            wq_sb = [
                wqk.tile([P, IC, P], bf16, tag=f"wq{m}", name=f"wq{m}" + sfx)
                for m in range(NPAIR)
            ]
            wk_sb = [
                wqk.tile([P, IC, P], bf16, tag=f"wk{m}", name=f"wk{m}" + sfx)
                for m in range(NPAIR)
            ]
            oTp = [
                oTp_pool.tile([P, S], bf16, tag=f"oTp{m}", name=f"oTp{m}" + sfx)
                for m in range(NPAIR)
            ]

            # ---- input DMAs (xt/wv interleaved: they gate the startup)
            for ic in range(IC):
                nc.sync.dma_start(xT_sb[:, ic, :], xt[P * ic : P * (ic + 1), :])
                nc.sync.dma_start(wv_sb[:, ic, :], wvt[P * ic : P * (ic + 1), :])
            for m in range(NPAIR):
                nc.sync.dma_start(
                    wq_sb[m][:],
                    wqt.rearrange("(ic p) o -> p ic o", p=P)[:, :, P * m : P * (m + 1)],
                )
                nc.sync.dma_start(
                    wk_sb[m][:],
                    wkt.rearrange("(ic p) o -> p ic o", p=P)[:, :, P * m : P * (m + 1)],
                )
            for m in range(NPAIR):
                nc.sync.dma_start(
                    woT_sb[:, m, :],
                    wot.rearrange("(m p) o -> p m o", p=P)[:, m, :],
                )

            # ---- emission helpers -------------------------------------
            def vproj_tile(t, pool=None):
                """V projection for seq tile t: natural [seq, head, dk+1]."""
                psv = (pool or pp_ps).tile([P, 512], f32, tag="pp", name=f"psv{t}" + sfx)
                for ic in range(IC):
                    nc.tensor.matmul(
                        psv[:],
                        xT_sb[:, ic, P * t : P * (t + 1)],
                        wv_sb[:, ic, :],
                        start=(ic == 0),
                        stop=(ic == IC - 1),
                    )
                nc.vector.tensor_add(
                    v_sb[t][:, :, 0:DK],
                    psv[:].rearrange("p (h d) -> p h d", d=DK),
                    bv_sb[:].rearrange("p (h d) -> p h d", d=DK),
                )
                nc.vector.memset(v_sb[t][:, :, DK : DK + 1], 1.0)

            def qkproj_chunk(m, c, which, pool=None):
                """q or k projection for pair m, query chunk c (256 wide)."""
                w_sb = wq_sb[m] if which == "q" else wk_sb[m]
                dst = qT[m] if which == "q" else kT[m]
                bias = bq_sb if which == "q" else bk_sb
                ps = (pool or pp_ps).tile(
                    [P, 512], f32, tag="pp", name=f"ps{which}{m}_{c}" + sfx
                )
                for ic in range(IC):
                    nc.tensor.matmul(
                        ps[:, 0:CH],
                        w_sb[:, ic, :],
                        xT_sb[:, ic, CH * c : CH * (c + 1)],
                        start=(ic == 0),
                        stop=(ic == IC - 1),
                    )
                nc.vector.tensor_scalar_add(
                    dst[:, CH * c : CH * (c + 1)],
                    ps[:, 0:CH],
                    bias[:, m : m + 1],
                )

            def scores_exp(m, c, jp):
                """Scores for key tiles j=2jp, 2jp+1 -> one pexp tile.

                pexp layout [P, 2(head), 2(jj), CH] bf16, key rows on
                partitions."""
                st = sc_ps.tile(
                    [P, 2, 2, CH], f32, tag="sc", name=f"st{m}_{c}_{jp}" + sfx
                )
                # one accumulation group per PSUM bank (bank h): start marks
                # the bank pending-zero, the jj=1 write lands on pending
                # bytes (overwrite), stop closes the group
                for jj in range(2):
                    j = 2 * jp + jj
                    off = max(0, P * j - CH * c)
                    for h in range(2):
                        nc.tensor.matmul(
                            st[:, h, jj, off:CH],
                            kT[m][64 * h : 64 * h + 64, P * j : P * (j + 1)],
                            qT[m][64 * h : 64 * h + 64, CH * c + off : CH * (c + 1)],
                            start=(jj == 0), stop=(jj == 1),
                        )
                pexp = pexp_pool.tile(
                    [P, 2, 2, CH], bf16, tag="pexp", name=f"pexp{m}_{c}_{jp}" + sfx
                )
                if jp == c:
                    # j=2c full width; j=2c+1 valid only in cols 128:256
                    nc.scalar.activation(pexp[:, :, 0, :], st[:, :, 0, :], AF.Exp)
                    nc.scalar.activation(
                        pexp[:, :, 1, P:CH], st[:, :, 1, P:CH], AF.Exp
                    )
                    # diagonal masks: j=2c at cols 0:128, j=2c+1 at cols 128:256
                    nc.vector.tensor_mul(
                        pexp[:, :, 0, 0:P],
                        pexp[:, :, 0, 0:P],
                        tri[:, None, :].to_broadcast((P, 2, P)),
                    )
                    nc.vector.tensor_mul(
                        pexp[:, :, 1, P:CH],
                        pexp[:, :, 1, P:CH],
                        tri[:, None, :].to_broadcast((P, 2, P)),
                    )
                else:
                    nc.scalar.activation(
                        pexp[:].rearrange("p a b q -> p (a b q)"),
                        st[:].rearrange("p a b q -> p (a b q)"),
                        AF.Exp,
                    )
                return pexp

            def attnv(m, c, jp, pexp, o_nat):
                """o_nat[:, qb, h] += pexp[:, h, jj, qb-block].T @ v_aug.

                o_nat is one PSUM tile [P, 2(qb), 2(h), 65] (1 bank)."""
                # o_nat is ONE psum bank: a single accumulation group wraps
                # all (qb, h) sub-regions — one start (marks bank pending-
                # zero; each region's first write overwrites), one stop on
                # the very last matmul into the bank
                for jj in range(2):
                    j = 2 * jp + jj
                    for qb in range(2):
                        t = 2 * c + qb
                        if t < j:
                            continue
                        for h in range(2):
                            nc.tensor.matmul(
                                o_nat[:, qb, h, :],
                                pexp[:, h, jj, P * qb : P * (qb + 1)],
                                v_sb[j][:, 2 * m + h, :],
                                start=(j == 0 and qb == 0 and h == 0),
                                stop=(j == 2 * c + 1 and qb == 1 and h == 1),
                            )

            def normalize(m, c, o_nat):
                """DVE: o_norm = o_nat[..., 0:64] / denominator (ones col)."""
                recip = recip_pool.tile(
                    [P, 2, 2], f32, tag="recip", name=f"rc{m}_{c}" + sfx
                )
                nc.vector.reciprocal(recip[:], o_nat[:, :, :, DK])
                o_norm = onorm_pool.tile(
                    [P, 2, 2, DK], bf16, tag="onorm", name=f"on{m}_{c}" + sfx
                )
                nc.vector.tensor_mul(
                    o_norm[:],
                    o_nat[:, :, :, 0:DK],
                    recip[:, :, :, None].to_broadcast((P, 2, 2, DK)),
                )
                return o_norm

            def transpose_onorm(m, c, qb, o_norm):
                """PE-transpose o_norm [128q, 128dims] -> oTp[m][:, tile]."""
                t = 2 * c + qb
                pt = ov_ps.tile([P, P], bf16, tag="ov", name=f"tf{m}_{c}_{qb}" + sfx)
                nc.tensor.transpose(
                    pt[:],
                    o_norm[:, qb].rearrange("p two d -> p (two d)"),
                    ident[:],
                )
                nc.vector.tensor_copy(oTp[m][:, P * t : P * (t + 1)], pt[:])

            def outproj(c, qb, fast=False):
                """Output projection for query tile t = 2c+qb (all pairs)."""
                t = 2 * c + qb
                for n in range(2):
                    psf = ov_ps.tile(
                        [P, 512], f32, tag="ov", name=f"fin{t}_{n}" + sfx
                    )
                    for m in range(NPAIR):
                        nc.tensor.matmul(
                            psf[:],
                            oTp[m][:, P * t : P * (t + 1)],
                            woT_sb[:, m, 512 * n : 512 * (n + 1)],
                            start=(m == 0),
                            stop=(m == NPAIR - 1),
                        )
                    fstg = stage_pool.tile(
                        [P, 512], f32, tag="fstg", name=f"fstg{t}_{n}" + sfx
                    )
                    nc.vector.tensor_copy(fstg[:], psf[:])
                    nc.sync.dma_start(
                        out[P * t : P * (t + 1), 512 * n : 512 * (n + 1)], fstg[:]
                    )

            # ---- schedule ---------------------------------------------
            # Startup phase: a dedicated 6-slot PSUM pool lets 6
            # accumulation groups track the xt/wv DMA fill; main attention
            # pools open after it closes.
            with ExitStack() as sctx:
                spool = sctx.enter_context(
                    tc.tile_pool(name="spool" + sfx, bufs=6, space="PSUM")
                )
                for t in range(6):
                    vproj_tile(t, pool=spool)
                for m in range(NPAIR):
                    qkproj_chunk(m, 0, "q", pool=spool)
                    qkproj_chunk(m, 0, "k", pool=spool)

            # PSUM (8 banks):
            #   sc: scores [P, 2, 2, 256] f32 (2 banks) x2     = 4
            #   ov: o_nat [P,2,2,65] f32 / transpose [P,128] bf16 /
            #       outproj psf [P,512] f32 (1 bank) x3         = 3
            #   pp: projections psq/psk/psv (1 bank) x1         = 1
            sc_ps = ctx0.enter_context(
                tc.tile_pool(name="scps" + sfx, bufs=2, space="PSUM")
            )
            ov_ps = ctx0.enter_context(
                tc.tile_pool(name="ovps" + sfx, bufs=3, space="PSUM")
            )
            pp_ps = ctx0.enter_context(
                tc.tile_pool(name="ppps" + sfx, bufs=1, space="PSUM")
            )

            # prev_onorm: (m, c, o_norm) awaiting transposes.
            # boundary: deferred outproj(c, qb) closures. Each allocates 2
            # ov-pool slots, so at most ONE may run inside a pair's j-loop
            # (between the o_nat alloc and the transposes); the rest run at
            # pair boundaries. Drained greedily in the Act-bound late chunks.
            prev_onorm = None
            boundary = []
            pop_budget = {5: 2, 6: 5, 7: 8}
            _head2 = True
            for c in range(NCH):
                # filler for chunk c+1 (pp-pool only -> safe inside j-loops).
                # chunk 7's own projections are NOT pre-run: they're emitted
                # just-in-time at chunk 7's pair heads (one pair ahead) so the
                # Act-bound last chunk has PE work co-located with its exps.
                filler = []
                if c + 1 < NCH - 1:
                    for m in range(NPAIR):
                        filler.append(lambda m=m: qkproj_chunk(m, c + 1, "q"))
                        filler.append(lambda m=m: qkproj_chunk(m, c + 1, "k"))
                elif c + 1 == NCH - 1:
                    filler.append(lambda: qkproj_chunk(0, NCH - 1, "q"))
                    filler.append(lambda: qkproj_chunk(0, NCH - 1, "k"))
                # v tiles two chunks ahead (t0-5 in startup); t14-15 are
                # deferred to chunk 7's pair heads
                for t in (2 * c + 6, 2 * c + 7):
                    if t < NT - 2:
                        filler.append(lambda t=t: vproj_tile(t))
                nfill = len(filler)
                nunits = NPAIR * (c + 1)
                ui = 0
                fi = 0
                pops = pop_budget.get(c, 0)

                head2 = _head2 and c >= 1
                for m in range(NPAIR):
                    # keep Act fed: first scores of this pair go out first
                    # (two of them with head2, so Act stays busy through the
                    # boundary pops below)
                    pexps = {0: scores_exp(m, c, 0)}
                    if head2:
                        pexps[1] = scores_exp(m, c, 1)
                    # previous pair's transposes: must precede this pair's
                    # o_nat allocation (ov-pool rotation order)
                    if prev_onorm is not None:
                        pm, pc, onorm = prev_onorm
                        for qb in range(2):
                            transpose_onorm(pm, pc, qb, onorm)
                        prev_onorm = None
                    # chunk 7: JIT projections for the NEXT pair + late V
                    # tiles, emitted at the pair head (fills exp latency)
                    if c == NCH - 1:
                        if m < NPAIR - 1:
                            qkproj_chunk(m + 1, c, "q")
                            qkproj_chunk(m + 1, c, "k")
                        if m == 0:
                            vproj_tile(NT - 2)
                            vproj_tile(NT - 1)
                    # deferred outproj at the pair boundary (any number of
                    # ov-slot allocations is rotation-safe here)
                    ntake = min(pops, (1 if m < 2 else 2), len(boundary))
                    for _ in range(ntake):
                        boundary.pop(0)()
                        pops -= 1
                    o_nat = ov_ps.tile(
                        [P, 2, 2, DK + 1], f32, tag="ov",
                        name=f"onat{m}_{c}" + sfx,
                    )
                    popped_inloop = False
                    for jp in range(c + 1):
                        nxt = jp + 2 if head2 else jp + 1
                        if nxt <= c and nxt not in pexps:
                            pexps[nxt] = scores_exp(m, c, nxt)
                        # filler before attnv: covers the exp latency
                        want = (nfill * (ui + 1)) // max(nunits, 1)
                        while fi < want:
                            filler[fi]()
                            fi += 1
                        # one deferred outproj inside the j-loop (2 ov slots)
                        if (
                            not popped_inloop and pops > 0 and boundary
                            and jp == min(1, c)
                        ):
                            boundary.pop(0)()
                            pops -= 1
                            popped_inloop = True
                        attnv(m, c, jp, pexps[jp], o_nat)
                        del pexps[jp]
                        ui += 1
                    o_norm = normalize(m, c, o_nat)
                    prev_onorm = (m, c, o_norm)
                while fi < nfill:
                    filler[fi]()
                    fi += 1
                if c < NCH - 1:
                    boundary.append(lambda c=c: outproj(c, 0))
                    boundary.append(lambda c=c: outproj(c, 1))

            # drain: last pair's transposes + remaining output projections
            pm, pc, onorm = prev_onorm
            for qb in range(2):
                transpose_onorm(pm, pc, qb, onorm)
            for fn in boundary:
                fn()
            for qb in range(2):
                outproj(NCH - 1, qb, fast=True)

    nc.compile()
    return nc


def get_program(reps=1):
    if reps not in _PROGRAM_CACHE:
        _PROGRAM_CACHE[reps] = build_program(reps)
    return _PROGRAM_CACHE[reps]


def make_in_maps(x, w_q, b_q, w_k, b_k, w_v, b_v, w_o):
    bf = ml_dtypes.bfloat16
    f8 = ml_dtypes.float8_e4m3fn
    x = np.asarray(x, np.float32)
    w_q = np.asarray(w_q, np.float32) * 0.125  # fold 1/sqrt(dk)
    w_k = np.asarray(w_k, np.float32)
    w_v = np.asarray(w_v, np.float32)
    w_o = np.asarray(w_o, np.float32)
    b_q = np.asarray(b_q, np.float32) * 0.125
    b_k = np.asarray(b_k, np.float32)
    b_v = np.asarray(b_v, np.float32)
    in_maps = []
    for core in range(8):
        b, g = divmod(core, 2)
        sl = slice(512 * g, 512 * (g + 1))
        in_maps.append(
            {
                "xt": np.ascontiguousarray(x[b].T).astype(bf),
                "wqt": np.ascontiguousarray(w_q[sl].T).astype(bf),
                "wkt": np.ascontiguousarray(w_k[sl].T).astype(bf),
                "wvt": np.ascontiguousarray(w_v[sl].T).astype(bf),
                "wot": np.ascontiguousarray(w_o[:, sl].T).astype(bf),
                "bqs": np.ascontiguousarray(b_q[sl].reshape(NPAIR, P).T),
                "bks": np.ascontiguousarray(b_k[sl].reshape(NPAIR, P).T),
                "bvb": np.ascontiguousarray(np.tile(b_v[sl][None, :], (P, 1))),
            }
        )
    return in_maps


def kernel(x, mask, w_q, b_q, w_k, b_k, w_v, b_v, w_o, b_o):
    from concourse.bass_utils import run_bass_kernel_spmd

    nc = get_program()
    in_maps = make_in_maps(x, w_q, b_q, w_k, b_k, w_v, b_v, w_o)
    res = run_bass_kernel_spmd(nc, in_maps, core_ids=list(range(8)), trace=False)
    b_o = np.asarray(b_o, np.float32)
    outs = []
    for b in range(4):
        outs.append(
            res.results[2 * b]["out"] + res.results[2 * b + 1]["out"] + b_o[None, :]
        )
    return np.stack(outs).astype(np.float32)
